# revision 60
# baseline (speedup 1.0000x reference)
"""Compressed MoE block on 8 Trainium2 NeuronCores.

Expert-parallel sharding: core e owns expert e. The router (tiny: T x H @
H x E) runs on host as part of dispatch; tokens are gathered per selected
expert (top-2), padded to a fixed capacity, and each core runs the full
factored FFN chain for its expert in token-transposed layout:

    g1T = Ug'(e).T @ xT          (Ug' = Ug @ Cg folded on host)
    gT  = Vg(e).T  @ g1T
    u1T = Uu'(e).T @ xT
    uT  = Vu(e).T  @ u1T
    aT  = silu(gT) * uT
    d1T = Ud'(e).T @ aT          (Ud' = Ud @ Cd)
    yT  = Vd(e).T  @ d1T

All matmul operands are fp16 (full-rate PE path, fp32 PSUM accumulation,
half the HBM traffic of fp32r). Inputs are host-packed into a
consumption-ordered stream striped across the two HWDGE rings
(sync/scalar) so descriptor-issue time and transfer order never pace
compute. Phase A runs as two m-waves of 4 PSUM accumulators each so
phase B's PSUM banks are free the moment its first matmuls are ready;
phase B's emission is software-pipelined one f-step ahead so the
silu->mul cross-engine latency never stalls the PE. Host scatters y back
with the renormalized top-2 routing weights.
"""

import numpy as np

import concourse.bacc as bacc
import concourse.mybir as mybir
import concourse.tile as tile
from concourse.bass_utils import run_bass_kernel_spmd

F32 = mybir.dt.float32
F16 = mybir.dt.float16

E = 8
KTOP = 2
H = 1024
FF = 2816
R = 256
KH = H // 128    # 8
KR = R // 128    # 2
KF = FF // 128   # 22
MH = H // 128    # 8

_BUILD_CACHE = {}
LAST_RESULT = None


def _build(C, nch):
    """Build the per-core bass program for capacity C split into nch chunks."""
    chunk = C // nch
    WB = 3 * R          # per-f block in wbuf: [vg_f | vu_f | udc_f]
    nc = bacc.Bacc()

    NFP = (KF + 1) // 2  # f-pair pieces
    AB0 = 2 * 128 + C    # per-k wave-1 block: [ugc_k_m0 | uuc_k_m0 | xt_k]
    # ab1: wave-1 critical stream, one block per k; wm1: wave-2 weights
    ab1 = nc.declare_dram_parameter("ab1", [KH, 128, AB0], F16, isOutput=False)
    wm1 = nc.declare_dram_parameter("wm1", [128, KH * 2 * 128], F16,
                                    isOutput=False)
    wbuf = nc.declare_dram_parameter("wbuf", [NFP, 128, 2 * WB], F16, isOutput=False)
    vdp = nc.declare_dram_parameter("vdp", [128, MH * R], F16, isOutput=False)
    ytp = nc.declare_dram_parameter("ytp", [128, MH * C], F16, isOutput=True)

    with tile.TileContext(nc) as tc:
        with (
            tc.tile_pool(name="wsb", bufs=1) as wsb,
            tc.tile_pool(name="work", bufs=3) as work,
            tc.tile_pool(name="pmm", bufs=8, space="PSUM") as pmm,
        ):
            a1s = wsb.tile([128, KH * AB0], F16, tag="a1s")
            w1s = wsb.tile([128, KH * 2 * 128], F16, tag="w1s")
            wb = wsb.tile([128, KF * WB], F16, tag="wb")
            vds = wsb.tile([128, MH * R], F16, tag="vds")
            g1s = wsb.tile([128, KR * C], F16, tag="g1s")
            u1s = wsb.tile([128, KR * C], F16, tag="u1s")
            d1s = wsb.tile([128, KR * C], F16, tag="d1s")
            warm0 = wsb.tile([128, 128], F32, tag="warm0")
            warm = wsb.tile([128, 128], F16, tag="warm")

            def ugc_k(k, m):
                if m == 0:
                    return a1s[:, k * AB0:k * AB0 + 128]
                return w1s[:, k * 256:k * 256 + 128]

            def uuc_k(k, m):
                if m == 0:
                    return a1s[:, k * AB0 + 128:k * AB0 + 256]
                return w1s[:, k * 256 + 128:(k + 1) * 256]

            def xt_k(k, c0):
                o = k * AB0 + 256 + c0
                return a1s[:, o:o + chunk]

            def vg_f(f, k):
                o = f * WB + k * 128
                return wb[:, o:o + 128]

            def vu_f(f, k):
                o = f * WB + R + k * 128
                return wb[:, o:o + 128]

            def udc_f(f, m):
                o = f * WB + 2 * R + m * 128
                return wb[:, o:o + 128]

            # --- PE warm-up: start the HAM activity window while DMAs land.
            # Full-duty 128-row matmuls maximize the activity integral so
            # the HAM 100%-utilization grant opens as early as possible.
            nc.vector.memset(warm0[:], 0.0)
            nc.vector.tensor_copy(warm[:], warm0[:])
            wps = pmm.tile([128, 128], F32, tag="mm", name="wps")
            NWARM = 22
            for i in range(NWARM):
                nc.tensor.matmul(
                    wps[:], warm[:], warm[:],
                    start=(i == 0), stop=(i == NWARM - 1),
                )

            # --- input DMAs: one consumption-ordered stream striped across
            # the two fast HWDGE rings (sync and scalar): the per-k wave-1
            # blocks [m0-weights | x] first (wave 1 is compute-paced from
            # the first block even at DVFS ramp clock), the wave-2 weights
            # next (split over both rings, landing before wave 2 starts),
            # then the phase-B weight pairs. Scalar's issues finish long
            # before its phase-B activation work starts.
            for k in range(KH):
                eng = (nc.sync, nc.scalar)[k % 2]
                eng.dma_start(a1s[:, k * AB0:(k + 1) * AB0], ab1[k])
            W1H = KH * 128
            nc.sync.dma_start(w1s[:, 0:W1H], wm1[:, 0:W1H])
            nc.scalar.dma_start(w1s[:, W1H:2 * W1H], wm1[:, W1H:2 * W1H])
            # Pre-load the scalar engine's Silu activation table right
            # after its (few) phase-A DMA issues: a lazy 1.3us
            # ACT_TABLE_LOAD otherwise lands on the critical path at phase
            # B's first silu.
            nc.scalar.activation(
                warm0[:, :16], warm0[:, :16],
                mybir.ActivationFunctionType.Silu,
            )
            # All phase-B weight pieces ride the sync ring: sync has no
            # compute role, while scalar must be free early for the
            # phase-boundary copies and silus (its DMA issues otherwise
            # drain until ~18us from ring-depth waits, gating phase B).
            # Single-ring delivery (~1.7us/piece) still outpaces phase B's
            # ~2.7us/piece consumption.
            for i in range(NFP):
                nc.sync.dma_start(wb[:, 2 * i * WB:2 * (i + 1) * WB], wbuf[i])
            nc.sync.dma_start(vds[:], vdp[:])

            # --- phase A: g1T/u1T [R, C] = Ug'/Uu'.T @ xT, in two m-waves of
            # 4 PSUM accumulators each (k-outer inside a wave so compute
            # paces the per-k input DMA stream). Wave copies overlap the
            # next wave's matmuls and phase B's PSUM banks are free early.
            for m in range(KR):
                psA = [
                    pmm.tile([128, chunk], F32, tag="mm", name=f"psA_{m}_{t}_{n}")
                    for t in range(2) for n in range(nch)
                ]
                for k in range(KH):
                    for t, wfun in enumerate((ugc_k, uuc_k)):
                        for n in range(nch):
                            nc.tensor.matmul(
                                psA[t * nch + n][:],
                                wfun(k, m),
                                xt_k(k, n * chunk),
                                start=(k == 0), stop=(k == KH - 1),
                            )
                # PSUM -> SBUF fp16 casts, split across vector/scalar, in
                # phase-B consumption order (g1 first).
                for ci, (t, dst) in enumerate(((0, g1s), (1, u1s))):
                    for n in range(nch):
                        c0 = n * chunk
                        dsl = dst[:, m * C + c0:m * C + c0 + chunk]
                        src = psA[t * nch + n][:]
                        if (ci * nch + n) % 2 == 0:
                            nc.vector.tensor_copy(dsl, src)
                        else:
                            nc.scalar.activation(
                                dsl, src, mybir.ActivationFunctionType.Copy
                            )

            # --- phase B: f-loop, both chunks per f (n-inner), fused
            # silu*up and d1 accumulation. Emission is software-pipelined by
            # one f-step: the PE runs gu(f+1) between ups(f) and d1(f), so
            # the cross-engine silu->mul latency never stalls the PE. The
            # PSUM pool (8 banks) cycles as gu(f)+gu(f+1), then d1p replaces
            # gu(f) once silu/mul free it; d1p accumulators are allocated
            # lazily at f==0 for the same reason.
            def emit_gu(f):
                gps = [
                    pmm.tile([128, chunk], F32, tag="mm", name=f"gps_{n}_{f}")
                    for n in range(nch)
                ]
                ups = [
                    pmm.tile([128, chunk], F32, tag="mm", name=f"ups_{n}_{f}")
                    for n in range(nch)
                ]
                # k-outer across both tensors: the k=0 step depends only on
                # wave-1 results, so at f==0 the PE crosses the phase A->B
                # boundary while wave-2's PSUM copies are still landing.
                for k in range(KR):
                    for ps, wfun, src in ((gps, vg_f, g1s), (ups, vu_f, u1s)):
                        for n in range(nch):
                            c0 = n * chunk
                            nc.tensor.matmul(
                                ps[n][:], wfun(f, k),
                                src[:, k * C + c0:k * C + c0 + chunk],
                                start=(k == 0), stop=(k == KR - 1),
                            )
                afs = []
                for n in range(nch):
                    gsil = work.tile([128, chunk], F32, tag="gsil", bufs=4)
                    nc.scalar.activation(
                        gsil[:], gps[n][:], mybir.ActivationFunctionType.Silu
                    )
                    af = work.tile(
                        [128, chunk], F16, tag="af", name=f"af_{n}_{f}", bufs=6
                    )
                    nc.vector.tensor_mul(af[:], gsil[:], ups[n][:])
                    afs.append(af)
                return afs

            d1p = None

            def emit_d1(f, afs):
                nonlocal d1p
                if d1p is None:
                    d1p = [
                        pmm.tile([128, chunk], F32, tag="mm", name=f"d1p_{n}_{m}")
                        for n in range(nch) for m in range(KR)
                    ]
                for m in range(KR):
                    for n in range(nch):
                        nc.tensor.matmul(
                            d1p[n * KR + m][:], udc_f(f, m), afs[n][:],
                            start=(f == 0), stop=(f == KF - 1),
                        )

            prev_afs = emit_gu(0)
            for f in range(1, KF):
                afs = emit_gu(f)
                emit_d1(f - 1, prev_afs)
                prev_afs = afs
            emit_d1(KF - 1, prev_afs)
            ci = 0
            for n in range(nch):
                c0 = n * chunk
                for m in range(KR):
                    dsl = d1s[:, m * C + c0:m * C + c0 + chunk]
                    src = d1p[n * KR + m][:]
                    if ci % 2 == 0:
                        nc.vector.tensor_copy(dsl, src)
                    else:
                        nc.scalar.activation(
                            dsl, src, mybir.ActivationFunctionType.Copy
                        )
                    ci += 1

            # --- phase C: yT [H, C] = Vd.T @ d1T; one output DMA per m-tile
            # on the rings that are idle by now.
            for m in range(MH):
                ypsl = [
                    pmm.tile([128, chunk], F32, tag="mm", name=f"yps_{n}_{m}")
                    for n in range(nch)
                ]
                for k in range(KR):
                    for n in range(nch):
                        c0 = n * chunk
                        nc.tensor.matmul(
                            ypsl[n][:],
                            vds[:, m * R + k * 128:m * R + (k + 1) * 128],
                            d1s[:, k * C + c0:k * C + c0 + chunk],
                            start=(k == 0), stop=(k == KR - 1),
                        )
                yts = work.tile([128, C], F16, tag="yts", bufs=4)
                for n in range(nch):
                    c0 = n * chunk
                    if (m * nch + n) % 2 == 0:
                        nc.vector.tensor_copy(yts[:, c0:c0 + chunk], ypsl[n][:])
                    else:
                        nc.scalar.activation(
                            yts[:, c0:c0 + chunk], ypsl[n][:],
                            mybir.ActivationFunctionType.Copy,
                        )
                out_eng = (nc.sync, nc.scalar)[m % 2]
                out_eng.dma_start(ytp[:, m * C:(m + 1) * C], yts[:])

    nc.finalize()
    return nc


def _pack_k(a, kt):
    """[kt*128, X] -> [128, kt, X] partition-tiled per k."""
    x = a.shape[1]
    return np.ascontiguousarray(a.reshape(kt, 128, x).transpose(1, 0, 2))


def _pack_fmajor(a, kt):
    """[kt*128, ft*128] -> [128, ft, kt*128]: f-major, k tiles adjacent."""
    ft = a.shape[1] // 128
    return np.ascontiguousarray(
        a.reshape(kt, 128, ft, 128).transpose(1, 2, 0, 3).reshape(128, ft, kt * 128)
    )


def kernel(hidden_states, gate_w, Ug, Cg, Vg, Uu, Cu, Vu, Ud, Cd, Vd):
    global LAST_RESULT
    hidden_states = np.asarray(hidden_states, dtype=np.float32)
    gate_w = np.asarray(gate_w, dtype=np.float32)
    b, s, h = hidden_states.shape
    x = hidden_states.reshape(-1, h)
    T = x.shape[0]

    # --- router (host; part of dispatch)
    logits = (x @ gate_w).astype(np.float64)
    lmax = logits.max(axis=-1, keepdims=True)
    p = np.exp(logits - lmax)
    p /= p.sum(axis=-1, keepdims=True)
    i1 = np.argmax(p, axis=-1)
    p1 = p[np.arange(T), i1]
    p_masked = p.copy()
    p_masked[np.arange(T), i1] = -np.inf
    i2 = np.argmax(p_masked, axis=-1)
    p2 = p[np.arange(T), i2]
    w1 = (p1 / (p1 + p2)).astype(np.float32)
    w2 = (p2 / (p1 + p2)).astype(np.float32)

    idx_e = []
    wgt_e = []
    for e in range(E):
        sel1 = np.nonzero(i1 == e)[0]
        sel2 = np.nonzero(i2 == e)[0]
        ids = np.concatenate([sel1, sel2])
        ws = np.concatenate([w1[sel1], w2[sel2]])
        idx_e.append(ids)
        wgt_e.append(ws)

    max_n = max(len(ids) for ids in idx_e)
    nch = max(1, -(-max_n // 512))
    chunk = -(-max_n // (nch * 4)) * 4
    C = nch * chunk

    key = (C, nch)
    if key not in _BUILD_CACHE:
        _BUILD_CACHE[key] = _build(C, nch)
    nc = _BUILD_CACHE[key]

    f16 = np.float16
    in_maps = []
    for e in range(E):
        ids = idx_e[e]
        xT = np.zeros((h, C), f16)
        xT[:, :len(ids)] = x[ids].T.astype(f16)
        ugc = (Ug[e] @ Cg).astype(f16)
        uuc = (Uu[e] @ Cu).astype(f16)
        udc = (Ud[e] @ Cd).astype(f16)
        # ab1: per-k wave-1 blocks [ugc_k_m0 | uuc_k_m0 | xt_k];
        # wm1: k-major [ugc_k_m1 | uuc_k_m1] wave-2 weights
        AB0 = 256 + C
        ab1 = np.zeros((KH, 128, AB0), f16)
        wm1 = np.zeros((128, KH * 256), f16)
        for k in range(KH):
            rows = slice(k * 128, (k + 1) * 128)
            ab1[k, :, 0:128] = ugc[rows, 0:128]
            ab1[k, :, 128:256] = uuc[rows, 0:128]
            ab1[k, :, 256:AB0] = xT[rows, :]
            wm1[:, k * 256:k * 256 + 128] = ugc[rows, 128:256]
            wm1[:, k * 256 + 128:(k + 1) * 256] = uuc[rows, 128:256]
        # wbuf: per-f blocks [vg_f | vu_f | udc_f], paired per piece
        wflat = np.concatenate(
            [
                _pack_fmajor(np.asarray(Vg[e], f16), KR),
                _pack_fmajor(np.asarray(Vu[e], f16), KR),
                _pack_k(udc, KF),
            ],
            axis=2,
        ).transpose(1, 0, 2)  # [KF, 128, WB]
        wbuf = np.ascontiguousarray(
            wflat.reshape(KF // 2, 2, 128, wflat.shape[2])
            .transpose(0, 2, 1, 3)
            .reshape(KF // 2, 128, -1)
        )  # [NFP, 128, 2*WB]
        in_maps.append({
            "ab1": ab1,
            "wm1": wm1,
            "wbuf": wbuf,
            "vdp": np.ascontiguousarray(
                _pack_fmajor(np.asarray(Vd[e], f16), KR).reshape(128, -1)
            ),

        })

    res = run_bass_kernel_spmd(nc, in_maps, list(range(E)))
    LAST_RESULT = res

    out = np.zeros((T, h), np.float32)
    for e in range(E):
        ids = idx_e[e]
        ytp = res.results[e]["ytp"]
        yT = ytp.reshape(128, MH, C).transpose(1, 0, 2).reshape(h, C).astype(np.float32)
        out[ids] += wgt_e[e][:, None] * yT[:, :len(ids)].T
    return out.reshape(b, s, h)
